# revision 18
# baseline (speedup 1.0000x reference)
"""Single-head causal attention (B=8, T=2048, H=1024, D=64) on 8 TRN2 NeuronCores.

Data-parallel over batch: one batch element per core, no collectives.

Per core, everything transposed so contractions land on partitions.

Input xt bf16 pre-laid [G=4, 128, hb, 512]: four dma_starts, column-
disjoint so the dependency tracker lets them overlap (partition-disjoint
halves of one tile serialize!): sync ring carries groups 0,2 and scalar
groups 1,3, so groups 0+1 stream concurrently and land together ~10us
after the triggers, then 2+3.  DMA streams share ~200-230 GB/s of
aggregate bandwidth (per-stream rate also scales with run length:
16KB runs ~165 GB/s, 8KB ~95-110), so adding a third stream dilutes
the first group's share and delays the projection start; each extra
dma_start on a ring costs ~1-3us of turnaround, so every ring carries
exactly two transfers.  Weights pre-packed [128, 8, 192]
([Wk | Wq | Wv] per h-block, 1/8 folded into Wq) on the gpsimd ring.

Projection per 512-column group as its DMA lands: packed [Wk | Wq]
stationary -> psum (k rows 0..63, q rows 64..127), ONE full-width
[128,512] cast into kqT, q DMA-shifted to partitions 0..63 on the
gpsimd ring (the sync ring is never head-of-line blocked by it; that
serialization caused the baseline's super-group transition bubble).
v proj -> vT cast -> 4 PE transposes into one psum tile -> one strided
copy into v_aug natural rows with a ones column.

Scores/AV run kb-major over two 1024-wide super-groups so consecutive
matmuls share one LDWEIGHTS (the ~100ns stationary load is never hidden
by the sequencer; 512-wide tiles pay it on every matmul).  Scores
sT[kj, qi] -> [128,1024] psum; exp on ScalarE (no max subtraction:
scores bounded ~+-4); diagonal blocks masked with gpsimd affine_select;
pT bf16.  AV: oT[d, qi] += [v[kb] | ones].T @ pT[kb]; the ones column
accumulates the softmax denominator in psum row 64 for free.

Each 512-wide output bank is normalized as soon as its accumulation
stops, with no SBUF<->SBUF DMA hops (see emit_norm); the final bank is
normalized in two column chunks so the post-last-AV tail is a ~1.5us
chain over 128 columns instead of ~5us over 512.

The schedule is one flat software-pipelined stream: 34 PE warm-up
matmuls cover group 0/1's DMA and ramp the HAM-gated PE clock (top
tier needs ~5us of continuous work and any idle drops it), then proj
g0/g1, super-group 0 scores two ahead of AVs with proj g2/g3 slotted
where their input has landed, then super-group 1.  PSUM: one 2-buf
4-bank pool (proj accs + all score tiles), one 4-buf pool (v-transpose
tiles + oT banks + broadcast rows), all 8 banks.
"""

import sys
from contextlib import ExitStack

if "/opt/trn_rl_repo" not in sys.path:
    sys.path.insert(0, "/opt/trn_rl_repo")

import numpy as np
import ml_dtypes

import concourse.bass as bass
import concourse.tile as tile
from concourse import bacc, mybir
from concourse.bass_utils import run_bass_kernel_spmd

B, T, H, D = 8, 2048, 1024, 64
N_CORES = 8
HB = H // 128  # 8 h-blocks
G = 4  # 512-wide input/projection groups
GW = T // G  # 512
KB = T // 128  # 16 key blocks
SG = 2  # compute super-groups
SGW = T // SG  # 1024

LINEARIZE = False
F32 = mybir.dt.float32
BF16 = mybir.dt.bfloat16


def build_kernel():
    nc = bacc.Bacc("TRN2", target_bir_lowering=False, debug=False, num_devices=N_CORES)

    xt_d = nc.dram_tensor("xt", [G, 128, HB, GW], BF16, kind="ExternalInput").ap()
    w_d = nc.dram_tensor("w", [128, HB, 3 * D], BF16, kind="ExternalInput").ap()
    out_d = nc.dram_tensor("out", [D, T], F32, kind="ExternalOutput").ap()

    with tile.TileContext(nc, linearize=LINEARIZE) as tc:
        _build(tc, xt_d, w_d, out_d)

    nc.compile()
    return nc


def _build(tc, xt_d, w_d, out_d):
    nc = tc.nc
    ctx = ExitStack()
    singles = ctx.enter_context(tc.tile_pool(name="singles", bufs=1))
    bigpool = ctx.enter_context(tc.tile_pool(name="bigpool", bufs=2, space="PSUM"))
    opool = ctx.enter_context(tc.tile_pool(name="opool", bufs=4, space="PSUM"))
    ppool = ctx.enter_context(tc.tile_pool(name="ppool", bufs=4))

    # ---- input DMAs: column-disjoint groups, two concurrent rings ----
    w_s = singles.tile([128, HB, 3 * D], BF16)
    xt_s = singles.tile([128, G, HB, GW], BF16)
    wu_s = singles.tile([128, 512], BF16, name="wu_s")
    nc.vector.memset(wu_s[:], 0.0)
    nc.gpsimd.dma_start(out=w_s[:], in_=w_d[:])
    for g, eng in enumerate([nc.sync, nc.scalar, nc.sync, nc.scalar]):
        eng.dma_start(
            out=xt_s[:, g].rearrange("p hb t -> p (hb t)"),
            in_=xt_d[g].rearrange("p hb t -> p (hb t)"),
        )

    wkq = w_s[:, :, 0:128]  # [Wk | Wq] stationary halves
    wv = w_s[:, :, 128:192]

    kqT = singles.tile([128, T], BF16)  # rows 0..63 kT, rows 64..127 q
    qlo = singles.tile([64, T], BF16)  # q DMA-shifted to partitions 0..63
    vT = singles.tile([64, T], BF16)

    v_aug = singles.tile([128, KB, 65], BF16)
    nc.gpsimd.memset(v_aug[:, :, 64:65], 1.0)
    identb = singles.tile([64, 64], BF16)
    nc.gpsimd.memset(identb[:], 0.0)
    nc.gpsimd.affine_select(
        out=identb[:], in_=identb[:], compare_op=mybir.AluOpType.not_equal,
        fill=1.0, base=0, pattern=[[-1, 64]], channel_multiplier=1,
    )

    ones64 = singles.tile([1, 64], F32, name="ones64")
    nc.vector.memset(ones64[:], 1.0)

    oT_s = singles.tile([64, T], F32)
    pt = {}  # (sg, kb) -> bf16 tile starting at column c0
    oT_b = {}  # bank base -> [65, 512] psum tile

    def emit_warmup(n):
        wu_ps = bigpool.tile([128, 512], F32, tag="big", name="warmup")
        for _ in range(n):
            nc.tensor.matmul(
                wu_ps[:], wu_s[:, 0:128], wu_s[:], start=True, stop=True
            )

    def emit_proj_kq(g):
        gcols = bass.ds(g * GW, GW)
        acc = bigpool.tile([128, GW], F32, tag="big", name=f"acc_kq_{g}")
        for hb in range(HB):
            nc.tensor.matmul(
                acc[:], wkq[:, hb, :], xt_s[:, g, hb, :],
                start=(hb == 0), stop=(hb == HB - 1),
            )
        nc.vector.tensor_copy(kqT[:, gcols], acc[:])
        # q shift: early groups on the gpsimd ring (empty queue), late
        # groups on sync (idle after the inputs; on gpsimd the queued
        # diagonal affine_selects would delay them into the sg1 start)
        qeng = nc.gpsimd if g < 2 else nc.sync
        qeng.dma_start(out=qlo[:, gcols], in_=kqT[64:128, gcols])

    def emit_proj_v(g):
        gcols = bass.ds(g * GW, GW)
        acc = bigpool.tile([64, GW], F32, tag="big", name=f"acc_v_{g}")
        for hb in range(HB):
            nc.tensor.matmul(
                acc[:], wv[:, hb, :], xt_s[:, g, hb, :],
                start=(hb == 0), stop=(hb == HB - 1),
            )
        nc.vector.tensor_copy(vT[:, gcols], acc[:])
        # natural v rows via PE transpose: 4 blocks into one psum tile,
        # then one strided copy into v_aug
        vtr = opool.tile([128, 4, 64], BF16, tag="o", name=f"vtr_{g}")
        for j in range(4):
            kb = 4 * g + j
            nc.tensor.transpose(vtr[:, j], vT[:, bass.ts(kb, 128)], identb[:])
        nc.vector.tensor_copy(v_aug[:, 4 * g : 4 * g + 4, 0:64], vtr[:])

    def emit_score(sg, kb):
        g0 = sg * SGW
        qi_lo = kb * 128
        c0 = max(qi_lo, g0)  # first causal column in this super-group
        t0 = g0 if c0 < g0 + 512 else g0 + 512  # tile base (bank-aligned)
        s_ps = bigpool.tile(
            [128, g0 + SGW - t0], F32, tag="big", name=f"s_{sg}_{kb}"
        )
        for b0 in range(t0, g0 + SGW, 512):
            m0 = max(c0, b0)
            nc.tensor.matmul(
                s_ps[:, m0 - t0 : b0 + 512 - t0],
                kqT[0:64, bass.ts(kb, 128)],
                qlo[:, bass.ds(m0, b0 + 512 - m0)],
                start=True,
                stop=True,
            )
        p = ppool.tile([128, g0 + SGW - c0], BF16, tag="pt", name=f"pt_{sg}_{kb}")
        pt[(sg, kb)] = p
        nc.scalar.activation(
            out=p[:],
            in_=s_ps[:, c0 - t0 :],
            func=mybir.ActivationFunctionType.Exp,
        )
        if c0 == qi_lo:
            # diagonal block: zero where kj (partition) > qi (free)
            nc.gpsimd.affine_select(
                out=p[:, 0:128],
                in_=p[:, 0:128],
                compare_op=mybir.AluOpType.is_ge,
                fill=0.0,
                base=0,
                pattern=[[1, 128]],
                channel_multiplier=-1,
            )

    def emit_av(sg, kb):
        g0 = sg * SGW
        c0 = max(kb * 128, g0)
        for b0 in range(g0, g0 + SGW, 512):
            m0 = max(c0, b0)
            if m0 >= b0 + 512:
                continue
            if b0 not in oT_b:
                oT_b[b0] = opool.tile([65, 512], F32, tag="o", name=f"oT_{b0}")
            nc.tensor.matmul(
                oT_b[b0][:, m0 - b0 : 512],
                v_aug[:, kb, :],
                pt[(sg, kb)][:, m0 - c0 : b0 + 512 - c0],
                start=(kb == 0),
                stop=(kb == 4 * (b0 // 512) + 3),
            )

    def emit_norm(b0, lo=0, hi=512):
        # normalize + store columns [lo:hi) of one 512-wide bank: psum
        # sums row -> DVE reciprocal_approx_fast (staged through SBUF) ->
        # broadcast to 64 partitions via PE f32 ones-outer-product (gpsimd
        # partition_broadcast lives in a different gpsimd library than
        # affine_select and the library reload stalls that engine ~7us)
        # -> stage to SBUF (DVE cannot read two PSUM operands) -> DVE
        # multiply -> DMA out on the sync ring.
        w = hi - lo
        bs = bass.ds(b0 + lo, w)
        srow = singles.tile([1, w], F32, name=f"srow_{b0}_{lo}")
        nc.vector.tensor_copy(srow[:], oT_b[b0][64:65, lo:hi])
        rcp = singles.tile([1, w], F32, name=f"rcp_{b0}_{lo}")
        nc.vector.reciprocal_approx_fast(out=rcp[:], in_=srow[:])
        rb_ps = opool.tile([64, w], F32, tag="o", name=f"rb_{b0}_{lo}")
        nc.tensor.matmul(rb_ps[:], ones64[:], rcp[:], start=True, stop=True)
        rb = singles.tile([64, w], F32, name=f"rbs_{b0}_{lo}")
        nc.vector.tensor_copy(rb[:], rb_ps[:])
        nc.vector.tensor_mul(oT_s[:, bs], oT_b[b0][0:64, lo:hi], rb[:])
        nc.sync.dma_start(out=out_d[:, bs], in_=oT_s[:, bs])

    # ---- flat schedule ----
    emit_warmup(34)
    emit_proj_kq(0)
    emit_proj_v(0)
    emit_proj_kq(1)
    emit_proj_v(1)
    # super-group 0: kb 0..7, scores two ahead of AVs; proj g2/g3 slotted
    # where their DMA (landing ~7us after g0/g1) is covered
    emit_score(0, 0)
    emit_score(0, 1)
    emit_score(0, 2)
    emit_av(0, 0)
    emit_score(0, 3)
    emit_av(0, 1)
    emit_score(0, 4)
    emit_av(0, 2)
    emit_score(0, 5)
    emit_av(0, 3)
    emit_norm(0)
    emit_proj_kq(2)
    emit_score(0, 6)
    emit_av(0, 4)
    emit_score(0, 7)
    emit_av(0, 5)
    emit_proj_v(2)
    emit_av(0, 6)
    emit_proj_kq(3)
    emit_av(0, 7)
    emit_norm(512)
    emit_proj_v(3)
    # super-group 1: kb 0..15
    emit_score(1, 0)
    emit_score(1, 1)
    for kb in range(2, KB):
        emit_score(1, kb)
        emit_av(1, kb - 2)
        if kb - 2 == 11:
            emit_norm(1024)
    emit_av(1, KB - 2)
    emit_av(1, KB - 1)
    emit_norm(1536, 0, 384)
    emit_norm(1536, 384, 512)

    ctx.close()


_NC_CACHE = {}


def _get_nc():
    if "nc" not in _NC_CACHE:
        _NC_CACHE["nc"] = build_kernel()
    return _NC_CACHE["nc"]


def make_in_maps(x, Wk, Wq, Wv):
    bf16 = ml_dtypes.bfloat16
    x = np.asarray(x, dtype=np.float32)
    wq = np.asarray(Wq, dtype=np.float32) / np.sqrt(np.float32(D))
    wk = np.asarray(Wk, dtype=np.float32)
    wv = np.asarray(Wv, dtype=np.float32)
    # [H, 192] = [Wk | Wq | Wv], then -> [128, HB, 192] (h = hb*128 + p)
    w = np.concatenate([wk, wq, wv], axis=1).astype(bf16)
    w = np.ascontiguousarray(w.reshape(HB, 128, 3 * D).transpose(1, 0, 2))
    in_maps = []
    for b in range(B):
        xt = x[b].T.astype(bf16)  # [H, T]
        # [G, 128, HB, GW]: per (group, partition) one 8KB run
        xt = np.ascontiguousarray(
            xt.reshape(HB, 128, G, GW).transpose(2, 1, 0, 3)
        )
        in_maps.append({"xt": xt, "w": w})
    return in_maps


def kernel(x, Wk, Wq, Wv, **_ignored):
    nc = _get_nc()
    in_maps = make_in_maps(x, Wk, Wq, Wv)
    res = run_bass_kernel_spmd(nc, in_maps, core_ids=list(range(N_CORES)))
    out = np.stack([res.results[b]["out"].T for b in range(B)])
    return out.astype(np.float32)


if __name__ == "__main__":
    x = np.random.randn(B, T, H).astype(np.float32)
    s = 1.0 / np.sqrt(H)
    Wk = np.random.uniform(-s, s, (H, D)).astype(np.float32)
    Wq = np.random.uniform(-s, s, (H, D)).astype(np.float32)
    Wv = np.random.uniform(-s, s, (H, D)).astype(np.float32)
    out = kernel(x=x, Wk=Wk, Wq=Wq, Wv=Wv)
    print("out shape:", out.shape, "finite:", np.isfinite(out).all())


# revision 19
# speedup vs baseline: 1.0363x; 1.0363x over previous
"""Single-head causal attention (B=8, T=2048, H=1024, D=64) on 8 TRN2 NeuronCores.

Data-parallel over batch: one batch element per core, no collectives.

Per core, everything transposed so contractions land on partitions.

Input xt bf16 pre-laid [G=4, 128, hb, 512]: four dma_starts, column-
disjoint so the dependency tracker lets them overlap (partition-disjoint
halves of one tile serialize!): sync ring carries groups 0,2 and scalar
groups 1,3, so groups 0+1 stream concurrently and land together ~10us
after the triggers, then 2+3.  DMA streams share ~200-230 GB/s of
aggregate bandwidth (per-stream rate also scales with run length:
16KB runs ~165 GB/s, 8KB ~95-110), so adding a third stream dilutes
the first group's share and delays the projection start; each extra
dma_start on a ring costs ~1-3us of turnaround, so every ring carries
exactly two transfers.  Weights pre-packed [128, 8, 192]
([Wk | Wq | Wv] per h-block, 1/8 folded into Wq) on the gpsimd ring.

Projection per 512-column group as its DMA lands: packed [Wk | Wq]
stationary -> psum (k rows 0..63, q rows 64..127), ONE full-width
[128,512] cast into kqT, q DMA-shifted to partitions 0..63 (groups 0/1
on the then-empty gpsimd ring, groups 2/3 on sync once the inputs have
drained — a shift queued behind other engine work head-of-line blocks
the next super-group's scores; that caused the original transition
bubble).
v proj -> vT cast -> 4 PE transposes into one psum tile -> one strided
copy into v_aug natural rows with a ones column.

Scores/AV run kb-major over two 1024-wide super-groups so consecutive
matmuls share one LDWEIGHTS (the ~100ns stationary load is never hidden
by the sequencer; 512-wide tiles pay it on every matmul).  Scores
sT[kj, qi] -> [128,1024] psum; exp on ScalarE (no max subtraction:
scores bounded ~+-4); diagonal blocks masked with gpsimd affine_select;
pT bf16.  AV: oT[d, qi] += [v[kb] | ones].T @ pT[kb]; the ones column
accumulates the softmax denominator in psum row 64 for free.

Each 512-wide output bank is normalized as soon as its accumulation
stops, with no SBUF<->SBUF DMA hops (see emit_norm); the final bank is
normalized in two column chunks so the post-last-AV tail is a ~1.5us
chain over 128 columns instead of ~5us over 512.

The schedule is one flat software-pipelined stream: 34 PE warm-up
matmuls cover group 0/1's DMA and ramp the HAM-gated PE clock (top
tier needs ~5us of continuous work and any idle drops it), then proj
g0/g1, super-group 0 scores two ahead of AVs with proj g2/g3 slotted
where their input has landed, then super-group 1.  PSUM: one 2-buf
4-bank pool (proj accs + all score tiles), one 4-buf pool (v-transpose
tiles + oT banks + broadcast rows), all 8 banks.
"""

import sys
from contextlib import ExitStack

if "/opt/trn_rl_repo" not in sys.path:
    sys.path.insert(0, "/opt/trn_rl_repo")

import numpy as np
import ml_dtypes

import concourse.bass as bass
import concourse.tile as tile
from concourse import bacc, mybir
from concourse.bass_utils import run_bass_kernel_spmd

B, T, H, D = 8, 2048, 1024, 64
N_CORES = 8
HB = H // 128  # 8 h-blocks
G = 4  # 512-wide input/projection groups
GW = T // G  # 512
KB = T // 128  # 16 key blocks
SG = 2  # compute super-groups
SGW = T // SG  # 1024

LINEARIZE = False
F32 = mybir.dt.float32
BF16 = mybir.dt.bfloat16


def build_kernel():
    nc = bacc.Bacc("TRN2", target_bir_lowering=False, debug=False, num_devices=N_CORES)

    xt_d = nc.dram_tensor("xt", [G, 128, HB, GW], BF16, kind="ExternalInput").ap()
    w_d = nc.dram_tensor("w", [128, HB, 3 * D], BF16, kind="ExternalInput").ap()
    out_d = nc.dram_tensor("out", [D, T], F32, kind="ExternalOutput").ap()

    with tile.TileContext(nc, linearize=LINEARIZE) as tc:
        _build(tc, xt_d, w_d, out_d)

    nc.compile()
    return nc


def _build(tc, xt_d, w_d, out_d):
    nc = tc.nc
    ctx = ExitStack()
    singles = ctx.enter_context(tc.tile_pool(name="singles", bufs=1))
    bigpool = ctx.enter_context(tc.tile_pool(name="bigpool", bufs=2, space="PSUM"))
    opool = ctx.enter_context(tc.tile_pool(name="opool", bufs=4, space="PSUM"))
    ppool = ctx.enter_context(tc.tile_pool(name="ppool", bufs=4))

    # ---- input DMAs: column-disjoint groups, two concurrent rings ----
    w_s = singles.tile([128, HB, 3 * D], BF16)
    xt_s = singles.tile([128, G, HB, GW], BF16)
    wu_s = singles.tile([128, 512], BF16, name="wu_s")
    nc.vector.memset(wu_s[:], 0.0)
    nc.gpsimd.dma_start(out=w_s[:], in_=w_d[:])
    for g, eng in enumerate([nc.sync, nc.scalar, nc.sync, nc.scalar]):
        eng.dma_start(
            out=xt_s[:, g].rearrange("p hb t -> p (hb t)"),
            in_=xt_d[g].rearrange("p hb t -> p (hb t)"),
        )

    wkq = w_s[:, :, 0:128]  # [Wk | Wq] stationary halves
    wv = w_s[:, :, 128:192]

    kqT = singles.tile([128, T], BF16)  # rows 0..63 kT, rows 64..127 q
    qlo = singles.tile([64, T], BF16)  # q DMA-shifted to partitions 0..63
    vT = singles.tile([64, T], BF16)

    v_aug = singles.tile([128, KB, 65], BF16)
    nc.gpsimd.memset(v_aug[:, :, 64:65], 1.0)
    identb = singles.tile([64, 64], BF16)
    nc.gpsimd.memset(identb[:], 0.0)
    nc.gpsimd.affine_select(
        out=identb[:], in_=identb[:], compare_op=mybir.AluOpType.not_equal,
        fill=1.0, base=0, pattern=[[-1, 64]], channel_multiplier=1,
    )

    ones64 = singles.tile([1, 64], F32, name="ones64")
    nc.vector.memset(ones64[:], 1.0)

    oT_s = singles.tile([64, T], F32)
    pt = {}  # (sg, kb) -> bf16 tile starting at column c0
    oT_b = {}  # bank base -> [65, 512] psum tile

    def emit_warmup(n):
        wu_ps = bigpool.tile([128, 512], F32, tag="big", name="warmup")
        for _ in range(n):
            nc.tensor.matmul(
                wu_ps[:], wu_s[:, 0:128], wu_s[:], start=True, stop=True
            )

    def emit_proj_kq(g):
        gcols = bass.ds(g * GW, GW)
        acc = bigpool.tile([128, GW], F32, tag="big", name=f"acc_kq_{g}")
        for hb in range(HB):
            nc.tensor.matmul(
                acc[:], wkq[:, hb, :], xt_s[:, g, hb, :],
                start=(hb == 0), stop=(hb == HB - 1),
            )
        nc.vector.tensor_copy(kqT[:, gcols], acc[:])
        # q shift: early groups on the gpsimd ring (empty queue), late
        # groups on sync (idle after the inputs; on gpsimd the queued
        # diagonal affine_selects would delay them into the sg1 start)
        qeng = nc.gpsimd if g < 2 else nc.sync
        qeng.dma_start(out=qlo[:, gcols], in_=kqT[64:128, gcols])

    def emit_proj_v(g):
        gcols = bass.ds(g * GW, GW)
        acc = bigpool.tile([64, GW], F32, tag="big", name=f"acc_v_{g}")
        for hb in range(HB):
            nc.tensor.matmul(
                acc[:], wv[:, hb, :], xt_s[:, g, hb, :],
                start=(hb == 0), stop=(hb == HB - 1),
            )
        nc.vector.tensor_copy(vT[:, gcols], acc[:])
        # natural v rows via PE transpose: 4 blocks into one psum tile,
        # then one strided copy into v_aug
        vtr = opool.tile([128, 4, 64], BF16, tag="o", name=f"vtr_{g}")
        for j in range(4):
            kb = 4 * g + j
            nc.tensor.transpose(vtr[:, j], vT[:, bass.ts(kb, 128)], identb[:])
        nc.vector.tensor_copy(v_aug[:, 4 * g : 4 * g + 4, 0:64], vtr[:])

    def emit_score(sg, kb):
        g0 = sg * SGW
        qi_lo = kb * 128
        c0 = max(qi_lo, g0)  # first causal column in this super-group
        t0 = g0 if c0 < g0 + 512 else g0 + 512  # tile base (bank-aligned)
        s_ps = bigpool.tile(
            [128, g0 + SGW - t0], F32, tag="big", name=f"s_{sg}_{kb}"
        )
        for b0 in range(t0, g0 + SGW, 512):
            m0 = max(c0, b0)
            nc.tensor.matmul(
                s_ps[:, m0 - t0 : b0 + 512 - t0],
                kqT[0:64, bass.ts(kb, 128)],
                qlo[:, bass.ds(m0, b0 + 512 - m0)],
                start=True,
                stop=True,
            )
        p = ppool.tile([128, g0 + SGW - c0], BF16, tag="pt", name=f"pt_{sg}_{kb}")
        pt[(sg, kb)] = p
        nc.scalar.activation(
            out=p[:],
            in_=s_ps[:, c0 - t0 :],
            func=mybir.ActivationFunctionType.Exp,
        )
        if c0 == qi_lo:
            # diagonal block: zero where kj (partition) > qi (free)
            nc.gpsimd.affine_select(
                out=p[:, 0:128],
                in_=p[:, 0:128],
                compare_op=mybir.AluOpType.is_ge,
                fill=0.0,
                base=0,
                pattern=[[1, 128]],
                channel_multiplier=-1,
            )

    def emit_av(sg, kb):
        g0 = sg * SGW
        c0 = max(kb * 128, g0)
        for b0 in range(g0, g0 + SGW, 512):
            m0 = max(c0, b0)
            if m0 >= b0 + 512:
                continue
            if b0 not in oT_b:
                oT_b[b0] = opool.tile([65, 512], F32, tag="o", name=f"oT_{b0}")
            nc.tensor.matmul(
                oT_b[b0][:, m0 - b0 : 512],
                v_aug[:, kb, :],
                pt[(sg, kb)][:, m0 - c0 : b0 + 512 - c0],
                start=(kb == 0),
                stop=(kb == 4 * (b0 // 512) + 3),
            )

    def emit_norm(b0, lo=0, hi=512):
        # normalize + store columns [lo:hi) of one 512-wide bank: psum
        # sums row -> DVE reciprocal_approx_fast (staged through SBUF) ->
        # broadcast to 64 partitions via PE f32 ones-outer-product (gpsimd
        # partition_broadcast lives in a different gpsimd library than
        # affine_select and the library reload stalls that engine ~7us)
        # -> stage to SBUF (DVE cannot read two PSUM operands) -> DVE
        # multiply -> DMA out on the sync ring.
        w = hi - lo
        bs = bass.ds(b0 + lo, w)
        srow = singles.tile([1, w], F32, name=f"srow_{b0}_{lo}")
        nc.vector.tensor_copy(srow[:], oT_b[b0][64:65, lo:hi])
        rcp = singles.tile([1, w], F32, name=f"rcp_{b0}_{lo}")
        nc.vector.reciprocal_approx_fast(out=rcp[:], in_=srow[:])
        rb_ps = opool.tile([64, w], F32, tag="o", name=f"rb_{b0}_{lo}")
        nc.tensor.matmul(rb_ps[:], ones64[:], rcp[:], start=True, stop=True)
        rb = singles.tile([64, w], F32, name=f"rbs_{b0}_{lo}")
        nc.vector.tensor_copy(rb[:], rb_ps[:])
        nc.vector.tensor_mul(oT_s[:, bs], oT_b[b0][0:64, lo:hi], rb[:])
        nc.sync.dma_start(out=out_d[:, bs], in_=oT_s[:, bs])

    # ---- flat schedule ----
    emit_warmup(34)
    emit_proj_kq(0)
    emit_proj_v(0)
    emit_proj_kq(1)
    emit_proj_v(1)
    # super-group 0: kb 0..7, scores two ahead of AVs; proj g2/g3 slotted
    # where their DMA (landing ~7us after g0/g1) is covered
    emit_score(0, 0)
    emit_score(0, 1)
    emit_score(0, 2)
    emit_av(0, 0)
    emit_score(0, 3)
    emit_av(0, 1)
    emit_score(0, 4)
    emit_av(0, 2)
    emit_score(0, 5)
    emit_av(0, 3)
    emit_norm(0)
    emit_proj_kq(2)
    emit_score(0, 6)
    emit_av(0, 4)
    emit_score(0, 7)
    emit_av(0, 5)
    emit_proj_v(2)
    emit_av(0, 6)
    emit_proj_kq(3)
    emit_av(0, 7)
    emit_norm(512)
    emit_proj_v(3)
    # super-group 1: kb 0..15
    emit_score(1, 0)
    emit_score(1, 1)
    for kb in range(2, KB):
        emit_score(1, kb)
        emit_av(1, kb - 2)
        if kb - 2 == 11:
            emit_norm(1024)
    emit_av(1, KB - 2)
    emit_av(1, KB - 1)
    emit_norm(1536, 0, 384)
    emit_norm(1536, 384, 512)

    ctx.close()


_NC_CACHE = {}


def _get_nc():
    if "nc" not in _NC_CACHE:
        _NC_CACHE["nc"] = build_kernel()
    return _NC_CACHE["nc"]


def make_in_maps(x, Wk, Wq, Wv):
    bf16 = ml_dtypes.bfloat16
    x = np.asarray(x, dtype=np.float32)
    wq = np.asarray(Wq, dtype=np.float32) / np.sqrt(np.float32(D))
    wk = np.asarray(Wk, dtype=np.float32)
    wv = np.asarray(Wv, dtype=np.float32)
    # [H, 192] = [Wk | Wq | Wv], then -> [128, HB, 192] (h = hb*128 + p)
    w = np.concatenate([wk, wq, wv], axis=1).astype(bf16)
    w = np.ascontiguousarray(w.reshape(HB, 128, 3 * D).transpose(1, 0, 2))
    in_maps = []
    for b in range(B):
        xt = x[b].T.astype(bf16)  # [H, T]
        # [G, 128, HB, GW]: per (group, partition) one 8KB run
        xt = np.ascontiguousarray(
            xt.reshape(HB, 128, G, GW).transpose(2, 1, 0, 3)
        )
        in_maps.append({"xt": xt, "w": w})
    return in_maps


def kernel(x, Wk, Wq, Wv, **_ignored):
    nc = _get_nc()
    in_maps = make_in_maps(x, Wk, Wq, Wv)
    res = run_bass_kernel_spmd(nc, in_maps, core_ids=list(range(N_CORES)))
    out = np.stack([res.results[b]["out"].T for b in range(B)])
    return out.astype(np.float32)


if __name__ == "__main__":
    x = np.random.randn(B, T, H).astype(np.float32)
    s = 1.0 / np.sqrt(H)
    Wk = np.random.uniform(-s, s, (H, D)).astype(np.float32)
    Wq = np.random.uniform(-s, s, (H, D)).astype(np.float32)
    Wv = np.random.uniform(-s, s, (H, D)).astype(np.float32)
    out = kernel(x=x, Wk=Wk, Wq=Wq, Wv=Wv)
    print("out shape:", out.shape, "finite:", np.isfinite(out).all())
